# revision 56
# baseline (speedup 1.0000x reference)
"""ColorHistogramLoss Trainium2 kernel (8 NeuronCores, data-parallel).

Strategy: shard batch (32 -> 4 per core). Each core streams its 25MB of
pixels through SBUF in [128, 2048] plane-tiles, computes HSV per pixel on
VectorE (fp32), and produces cumulative histogram-edge counts
C(e) = #{x < e} for 9 edges x 3 components x (real, fake) via fused
tensor_scalar(is_lt, accum_out=...) ops.  Per-(iteration, edge)
per-partition counts are DMA'd out ([8*128, 32] per core); the host sums
partitions/cores, differences cumulative counts into 10-bin histograms and
computes the scalar loss.  All on-device count arithmetic is exact in f32.
"""

import sys

if "/opt/trn_rl_repo" not in sys.path:
    sys.path.insert(0, "/opt/trn_rl_repo")

import numpy as np

from concourse import bacc, mybir, tile
from concourse import bass_utils

# ---- problem constants (hardcoded; kernel.py must be self-contained) ----
B, C, H, W = 32, 3, 512, 512
NCORES = 8
BPC = B // NCORES            # batches per core
P, F = 128, 2048             # SBUF tile: one [512,512] plane = [128, 2048]
NITER = 2 * BPC              # 4 real + 4 fake plane-triple iterations
NEDGE = 26                   # acc slots: 12 hue-case + 9 val + 4 sat-dual + 1 sat
ACCW = 32                    # padded accumulator width
NPIX = B * H * W             # pixels per full histogram
ALPHA, BETA, GAMMA = 0.3, 0.4, 0.4

AF = mybir.AluOpType
F32 = mybir.dt.float32

LAST_EXEC_NS = None
_CACHE = {}

PACK = 4096.0  # EDGE2* dual-count packing: accum = cntA + PACK*cntB (exact in f32)

# Hue edge counting runs on per-case shifted values (shift=8 keeps ulp tiny):
#   A2 = hA + 8*(mb+mg')  (r-case in range, others at ~[7,9])
#   B2 = hB - 8*mg'       (g-case at [-9,-7], others in [-1,1])
#   C2 = hC - 8*mb        (b-case at [-9,-7], others in [-1,1])
# where hA=u/d, hB=v/d, hC=w/d.  Case totals R, G come free from the
# mask-op accumulators.  Slot edges (sign-counted on ScalarE):
HUE_EDGES = (
    -0.6, 0.0, 0.6,                 # A2: NA(-0.6), NEG, NA(0.6)
    -8.8, -8.2, -7.6, -7.0,         # B2: NB(e-10) for e=1.2..3.0
    -8.4, -7.8, -7.2,               # C2: NC(e-12) for e=3.6..4.8
)


def _register_custom_ops():
    """Author + register fused DVE ops in the dve_ops registry at runtime
    (the repo list is read-only; registration is by-name so appending to the
    module-level OPS list is sufficient for table-gen and tracing)."""
    from concourse import dve_ops
    from concourse.dve_spec import (
        C0, C1, C2, Spec, Src0, Src1, Zero, _has_src1, lower, maxx,
    )
    from concourse.dve_uop import DveOpSpec

    if hasattr(dve_ops, "HUE_MOD6"):
        return dve_ops

    _y = Src0 * Src1

    def _ref_hue_mod6(in0, in1, c0, c1, c2):
        y = in0.astype(np.float32) * in1
        return (y + c0 * (y < 0)).astype(np.float32)

    def _ref_abs2max(in0, in1, c0, c1, c2):
        return np.maximum(np.abs(in0.astype(np.float32)), np.abs(in1)).astype(
            np.float32
        )

    def _ref_absmax3(in0, in1, c0, c1, c2):
        return np.maximum(in0.astype(np.float32), np.abs(in1)).astype(np.float32)

    def _ref_edge2d(in0, in1, c0, c1, c2):
        b = ((in0.astype(np.float32) * c0 > in1) + c1 * (in0 * c2 > in1)).astype(
            np.float32
        )
        return b, b.reshape(b.shape[0], -1).sum(axis=-1, keepdims=True)

    from operator import add as _add

    defs = [
        # out = y + c0*(y<0), y = in0*in1   (hue mod-6 wrap, fused)
        ("HUE_MOD6", Spec(body=_y + C0 * (_y < Zero), reference=_ref_hue_mod6)),
        # out = max(|in0|, |in1|)
        (
            "ABS2MAX",
            Spec(
                body=maxx(maxx(Src0, Zero - Src0), maxx(Src1, Zero - Src1)),
                reference=_ref_abs2max,
            ),
        ),
        # out = max(in0, |in1|)
        (
            "ABSMAX3",
            Spec(
                body=maxx(Src0, maxx(Src1, Zero - Src1)),
                reference=_ref_absmax3,
            ),
        ),
        # dual sat-edge count: accum = #{in0*c0 > in1} + c1*#{in0*c2 > in1}
        (
            "EDGE2D",
            Spec(
                body=(Src0 * C0 > Src1) + C1 * ((Src0 * C2) > Src1),
                accum=_add,
                accum_init=Zero,
                reference=_ref_edge2d,
            ),
        ),
        # dual edge count: accum = #{in0 < c0} + c1*#{in0 < c2}
        (
            "EDGE2",
            Spec(
                body=(Src0 < C0) + C1 * (Src0 < C2),
                accum=_add,
                accum_init=Zero,
                reference=lambda in0, in1, c0, c1, c2: (
                    lambda b: (b, b.reshape(b.shape[0], -1).sum(-1, keepdims=True))
                )(((in0 < c0) + c1 * (in0 < c2)).astype(np.float32)),
            ),
        ),
        # mb = (in0 >= 0) & (in1 <= 0); accum = count  (in0=v, in1=u)
        (
            "MBC",
            Spec(
                body=(Src0 >= Zero) & (Src1 <= Zero),
                accum=_add,
                accum_init=Zero,
                reference=lambda in0, in1, c0, c1, c2: (
                    lambda b: (b, b.reshape(b.shape[0], -1).sum(-1, keepdims=True))
                )(((in0 >= 0) & (in1 <= 0)).astype(np.float32)),
            ),
        ),
        # nmg = -[(in0 > 0) & (in1 <= 0)]; accum = -count  (in0=u, in1=w)
        (
            "NMGC",
            Spec(
                body=Zero - ((Src0 > Zero) & (Src1 <= Zero)),
                accum=_add,
                accum_init=Zero,
                reference=lambda in0, in1, c0, c1, c2: (
                    lambda b: (b, b.reshape(b.shape[0], -1).sum(-1, keepdims=True))
                )((-((in0 > 0) & (in1 <= 0))).astype(np.float32)),
            ),
        ),
    ]
    for name, spec in defs:
        row = 1 + len(dve_ops.OPS)
        shas = {}
        for ver in ("v3", "v4"):
            uops = lower(spec, ver=ver)
            shas[ver] = DveOpSpec(
                name=name, opcode=row, uops=uops, rd1_en=_has_src1(spec)
            ).sha(ver)
        op = dve_ops.DveOp(name, spec, False, uops_sha=shas)
        dve_ops.OPS.append(op)
        dve_ops.CUSTOM_DVE_SPECS[name] = spec
        dve_ops._SUB_OPCODE_FOR_NAME[name] = row
        setattr(dve_ops, name, op)
    return dve_ops


def _build():
    dve_ops = _register_custom_ops()
    nc = bacc.Bacc(
        "TRN2", target_bir_lowering=False, debug=False, num_devices=NCORES
    )
    xr = nc.dram_tensor("x_real", [BPC * C * P, F], F32, kind="ExternalInput").ap()
    xf = nc.dram_tensor("x_fake", [BPC * C * P, F], F32, kind="ExternalInput").ap()
    out = nc.dram_tensor("out", [NITER * P, ACCW], F32, kind="ExternalOutput").ap()

    with tile.TileContext(nc) as tc:
        with tc.tile_pool(name="main", bufs=2) as io_pool, tc.tile_pool(
            name="tmp", bufs=1
        ) as tmp_pool:
            # per-edge bias tiles for ScalarE Sign activations (bias = -edge)
            ebias = []
            for k in range(1, 10):
                bt = tmp_pool.tile([P, 1], F32, tag=f"eb{k}", name=f"eb{k}")
                nc.gpsimd.memset(bt[:], -(0.1 * k))
                ebias.append(bt)
            # hue case-edges on shifted per-case values A2/B2/C2
            hedges = HUE_EDGES
            hbias = []
            for idx, e in enumerate(hedges):
                ht = tmp_pool.tile([P, 1], F32, tag=f"hb{idx}", name=f"hb{idx}")
                nc.gpsimd.memset(ht[:], -e)
                hbias.append(ht)
            for it in range(NITER):
                src = xr if it < BPC else xf
                bi = it % BPC

                def plane(c):
                    q = bi * C + c
                    return src[q * P : (q + 1) * P, :]

                r = io_pool.tile([P, F], F32, tag="r")
                g = io_pool.tile([P, F], F32, tag="g")
                bl = io_pool.tile([P, F], F32, tag="bl")
                # bl, r first: the opening VectorE op (v = bl - r) needs them
                nc.sync.dma_start(bl[:], plane(2))
                nc.sync.dma_start(r[:], plane(0))
                nc.sync.dma_start(g[:], plane(1))

                # double-buffer the tiles ScalarE reads across iterations
                # (mx=t1, A2=t2, B2=t4, C2=t5, d=t10) to break WAR stalls
                t = [
                    tmp_pool.tile(
                        [P, F], F32, tag=f"t{i}", name=f"t{i}",
                        bufs=2 if i in (1, 2, 4, 5, 10) else 1,
                    )
                    for i in range(11)
                ]
                V = nc.vector

                u = t[2]
                V.tensor_tensor(u[:], g[:], bl[:], AF.subtract)
                v = t[4]
                V.tensor_tensor(v[:], bl[:], r[:], AF.subtract)
                w = t[5]
                V.tensor_tensor(w[:], r[:], g[:], AF.subtract)
                # d = mx - mn == max(|u|, |v|, |w|) (exact: same fl-subtracts)
                d2 = t[3]
                V._custom_dve(dve_ops.ABS2MAX, out=d2[:], in0=u[:], in1=v[:])
                d = t[10]
                V._custom_dve(dve_ops.ABSMAX3, out=d[:], in0=d2[:], in1=w[:])
                rd = t[3]
                V.reciprocal_approx_fast(rd[:], d[:])
                acc = io_pool.tile([P, 19], F32, tag="acc")
                accv = io_pool.tile([P, 7], F32, tag="accv")
                scr = t[9]
                scr2 = tmp_pool.tile([P, F], F32, tag="scr2", name="scr2")
                SIGN = mybir.ActivationFunctionType.Sign
                mb = t[6]
                # mb = (v>=0)&(u<=0) == (mx==bl); accum -> count(mb)
                V._custom_dve(
                    dve_ops.MBC, out=mb[:], in0=v[:], in1=u[:],
                    accum_out=accv[:, 5:6],
                )
                nmg = t[8]
                # nmg = -[(u>0)&(w<=0)] == -[mg & !mb]; accum -> -count(mg')
                V._custom_dve(
                    dve_ops.NMGC, out=nmg[:], in0=u[:], in1=w[:],
                    accum_out=accv[:, 6:7],
                )
                s8 = t[7]
                V.tensor_tensor(s8[:], mb[:], nmg[:], AF.subtract)  # mb + mg'
                hA = t[9]
                V.tensor_tensor(hA[:], u[:], rd[:], AF.mult)
                A2 = t[2]
                V.scalar_tensor_tensor(A2[:], s8[:], 8.0, hA[:], AF.mult, AF.add)
                hBp = t[9]
                V.tensor_tensor(hBp[:], v[:], rd[:], AF.mult)
                B2 = t[4]
                V.scalar_tensor_tensor(B2[:], nmg[:], 8.0, hBp[:], AF.mult, AF.add)
                hCp = t[9]
                V.tensor_tensor(hCp[:], w[:], rd[:], AF.mult)
                C2 = t[5]
                V.scalar_tensor_tensor(C2[:], mb[:], -8.0, hCp[:], AF.mult, AF.add)
                # mx late: only the val/sat masks consume it
                m1, mx = t[0], t[1]
                V.tensor_tensor(m1[:], r[:], g[:], AF.max)
                V.tensor_tensor(mx[:], m1[:], bl[:], AF.max)
                # hue case-edge counts on ScalarE, sign-style: slots 0..9
                # accum = sum(Sign(x - e)); host decodes N_lt = (N - S)/2
                case_tiles = [A2] * 3 + [B2] * 4 + [C2] * 3
                for idx in range(10):
                    nc.scalar.activation(
                        scr2[:], case_tiles[idx][:], SIGN, bias=hbias[idx][:],
                        accum_out=acc[:, idx : idx + 1],
                    )
                if it < NITER - 1:
                    # val masks on ScalarE: slots 10..18 (sign-style)
                    for k in range(1, 10):
                        nc.scalar.activation(
                            scr2[:], mx[:], SIGN, bias=ebias[k - 1][:],
                            accum_out=acc[:, 9 + k : 10 + k],
                        )
                else:
                    # last iteration: run val masks on VectorE (EDGE2 duals)
                    # so ScalarE isn't the pipeline tail. Direct counts,
                    # flagged for the host by writing them as negatives
                    # minus one... (decoded by slot style table instead)
                    for j in range(4):
                        V._custom_dve(
                            dve_ops.EDGE2,
                            out=scr[:],
                            in0=mx[:],
                            s0=0.1 * (2 * j + 1),
                            s1=PACK,
                            imm2=0.1 * (2 * j + 2),
                            accum_out=acc[:, 10 + j : 11 + j],
                        )
                    V.tensor_scalar(
                        scr[:], mx[:], 0.9, None, AF.is_lt, AF.add,
                        accum_out=acc[:, 14:15],
                    )
                # sat masks on VectorE: dual-edge fused counts, accv 0..3
                # slot = #{0.1(2j+1)*mx > d} + PACK * #{0.1(2j+2)*mx > d}
                for j in range(4):
                    V._custom_dve(
                        dve_ops.EDGE2D,
                        out=scr[:],
                        in0=mx[:],
                        in1=d[:],
                        s0=0.1 * (2 * j + 1),
                        s1=PACK,
                        imm2=0.1 * (2 * j + 2),
                        accum_out=accv[:, j : j + 1],
                    )
                # 9th sat edge: direct single count, accv 4
                V.scalar_tensor_tensor(
                    scr[:], mx[:], 0.9, d[:], AF.mult, AF.is_gt,
                    accum_out=accv[:, 4:5],
                )
                nc.sync.dma_start(out[it * P : (it + 1) * P, 0:19], acc[:, :])
                nc.sync.dma_start(out[it * P : (it + 1) * P, 19:26], accv[:, :])

    nc.compile()
    return nc


def _register_ntff_hook():
    """Register the axon NTFF profiling hook (the container's antenv stub
    lacks axon_hooks, so trn_boot's registration was skipped). Also keep
    profile artifacts local instead of uploading to a share."""
    import types

    import antenv

    if "antenv.axon_hooks" not in sys.modules:
        mod = types.ModuleType("antenv.axon_hooks")
        holder = [None]
        mod.set_axon_ntff_profile_hook = lambda h: holder.__setitem__(0, h)
        mod.get_axon_ntff_profile_hook = lambda: holder[0]
        sys.modules["antenv.axon_hooks"] = mod
        antenv.axon_hooks = mod
    from antenv import axon_hooks

    if axon_hooks.get_axon_ntff_profile_hook() is None:
        from trn_agent_boot.trn_boot import _ntff_profile_via_ctypes

        axon_hooks.set_axon_ntff_profile_hook(
            _ntff_profile_via_ctypes("/opt/axon/libaxon_pjrt.so")
        )
    bass_utils.upload_artifacts = lambda tmpdir: tmpdir


def _get_nc():
    if "nc" not in _CACHE:
        _CACHE["nc"] = _build()
    return _CACHE["nc"]


def kernel(x_real: np.ndarray, x_fake: np.ndarray) -> np.ndarray:
    global LAST_EXEC_NS
    nc = _get_nc()

    in_maps = []
    for c in range(NCORES):
        sl = slice(c * BPC, (c + 1) * BPC)
        in_maps.append(
            {
                "x_real": np.ascontiguousarray(x_real[sl]).reshape(BPC * C * P, F),
                "x_fake": np.ascontiguousarray(x_fake[sl]).reshape(BPC * C * P, F),
            }
        )

    import os

    trace = bool(int(os.environ.get("KERNEL_TRACE", "0")))
    if trace:
        _register_ntff_hook()
    res = bass_utils.run_bass_kernel_spmd(
        nc, in_maps, core_ids=list(range(NCORES)), trace=trace
    )
    LAST_EXEC_NS = res.exec_time_ns
    _CACHE["last_res"] = res

    # Cols 0:10 hue case-edges + 10:19 val: sign-sums S = cnt_gt - cnt_lt,
    # decoded as N_lt = (N - S)/2.  Cols 19:23: packed dual sat counts
    # cntA + PACK*cntB (direct C_lt); col 23: sat edge 9; col 24: count(mb);
    # col 25: -count(mg').
    sign_sums = np.zeros((2, 19), np.float64)
    sat_C = np.zeros((2, 9), np.float64)
    val_direct = np.zeros(9, np.float64)  # last iteration's val counts (on V)
    caseB = np.zeros(2)
    caseG = np.zeros(2)
    for core_out in res.results:
        o = np.asarray(core_out["out"]).reshape(NITER, P, ACCW)
        for t_idx, sl in ((0, slice(0, BPC)), (1, slice(BPC, NITER))):
            blk = o[sl]
            sign_sums[t_idx, :10] += blk[:, :, :10].sum(axis=(0, 1))
            if t_idx == 0:
                sign_sums[0, 10:19] += blk[:, :, 10:19].sum(axis=(0, 1))
            else:
                # fake: iters 4..6 sign-style; iter 7 direct EDGE2-packed
                sign_sums[1, 10:19] += blk[:-1, :, 10:19].sum(axis=(0, 1))
                vp = blk[-1, :, 10:14].astype(np.int64)
                val_direct[0:8:2] += (vp % int(PACK)).sum(axis=0)
                val_direct[1:8:2] += (vp // int(PACK)).sum(axis=0)
                val_direct[8] += blk[-1, :, 14].sum()
            packed = blk[:, :, 19:23].astype(np.int64)  # exact ints in f32
            sat_C[t_idx, 0:8:2] += (packed % int(PACK)).sum(axis=(0, 1))
            sat_C[t_idx, 1:8:2] += (packed // int(PACK)).sum(axis=(0, 1))
            sat_C[t_idx, 8] += blk[:, :, 23].sum()
            caseB[t_idx] += blk[:, :, 24].sum()
            caseG[t_idx] -= blk[:, :, 25].sum()
    NL = (NPIX - sign_sums) / 2.0  # [2, 19] lt-counts per slot
    # fake val: sign part covers only (BPC-1)/BPC of the pixels
    NL[1, 10:19] = (NPIX * (BPC - 1) / BPC - sign_sums[1, 10:19]) / 2.0 + val_direct
    NA, NB, NC = NL[:, 0:3], NL[:, 3:7], NL[:, 7:10]
    NEG = NA[:, 1]
    R = NPIX - caseB - caseG
    C_lt = np.zeros((2, 3, 9), np.float64)
    # hue: reassemble cumulative counts from per-case counts
    C_lt[:, 0, 0] = NA[:, 2] - NEG                       # C(0.6)
    for j in range(4):                                   # C(1.2)..C(3.0)
        C_lt[:, 0, 1 + j] = (R - NEG) + NB[:, j]
    for j in range(3):                                   # C(3.6)..C(4.8)
        C_lt[:, 0, 5 + j] = (R - NEG) + caseG + NC[:, j]
    C_lt[:, 0, 8] = (NPIX - NEG) + NA[:, 0]              # C(5.4)
    C_lt[:, 2, :] = NL[:, 10:19]                         # val
    C_lt[:, 1, :] = sat_C

    hist = np.zeros((2, 3, 10), np.float64)
    hist[:, :, 0] = C_lt[:, :, 0]
    hist[:, :, 1:9] = C_lt[:, :, 1:] - C_lt[:, :, :-1]
    hist[:, :, 9] = NPIX - C_lt[:, :, 8]

    dmean = np.abs(hist[0] - hist[1]).mean(axis=1)   # [3] = h, s, v
    loss = ALPHA * dmean[0] + BETA * dmean[1] + GAMMA * dmean[2]
    return np.asarray(loss, dtype=np.float32)
